# revision 3
# baseline (speedup 1.0000x reference)
"""Trainium2 Bass kernel for the MoE bottleneck block.

Computation (per image):
    out1 = relu(bn1(conv1x1_256->64(x)))
    gate = relu(emb @ gate_w.T + gate_b)            (computed host-side, tiny)
    out2 = relu(bn2(conv3x3_64->64(out1) * gate[b,:,None,None]))
    out  = relu(bn3(conv1x1_64->256(out2)) + x)
    returns (out, gate)

Sharding: pure data parallel, 4 images per core on 8 cores.

Per-core schedule: each image is processed in 28-row horizontal slabs.
The x slab stays resident in SBUF so the residual add reuses the same
bytes loaded for conv1 (x is read from HBM once, not twice).

All BN scales are folded into conv weights on the host; gate*bn2_scale is
folded into per-image w2 copies.  Biases + relus are fused into the
PSUM->SBUF evacuation ops (ACT/DVE).

Matmuls run with 16-bit (or float32r) operands at full PE rate, f32 PSUM
accumulation (fp32 matmul is 1/4 rate on TRN2 and would be the
bottleneck).  The residual add rides the PE too: an identity matmul
accumulates x into conv3's PSUM group, so stage-C evacuation is a single
fused bias+relu op.

conv3x3 runs as 6 accumulated matmuls per 4-row chunk: the (kh=0,kh=1)
taps are packed into one K=128 matmul using a duplicated, one-row-shifted
copy of the padded out1 image living on SBUF partitions 64..127; kh=2
taps are K=64 matmuls.
"""

import sys

if "/opt/trn_rl_repo" not in sys.path:
    sys.path.insert(0, "/opt/trn_rl_repo")

import numpy as np

EPS = 1e-5

B, CIN, H, W = 32, 256, 112, 112
MID, COUT = 64, 256
N_CORES = 8
BPC = B // N_CORES            # images per core
SLAB = 28                     # output rows per slab
NSLABS = H // SLAB            # 4
PW = W + 2                    # padded width: 114
PROWS = SLAB + 2              # padded rows held per slab: 30

MM_DTYPE = "float16"          # float16 | bfloat16 | float32r | float32

_cache = {}


def _chunks(lo, hi, step):
    out = []
    h = lo
    while h < hi:
        n = min(step, hi - h)
        out.append((h, n))
        h += n
    return out


def build_nc(repeat=1, mm_dtype=None):
    """Build + compile the per-core Bass program. Same program on all cores."""
    import concourse.bacc as bacc
    import concourse.mybir as mybir
    import concourse.tile as tile

    mm_dtype = mm_dtype or MM_DTYPE
    f32 = mybir.dt.float32
    Act = mybir.ActivationFunctionType
    Alu = mybir.AluOpType

    # storage dtype of matmul operands in SBUF / DRAM weight tensors
    if mm_dtype == "float32r":
        sdt = f32
        mmdt = mybir.dt.float32r
    else:
        sdt = getattr(mybir.dt, mm_dtype)
        mmdt = sdt
    cast_load = sdt != f32  # x load needs a casting (SWDGE) DMA

    def mm_ap(ap):
        return ap.bitcast(mmdt) if mmdt != sdt else ap

    nc = bacc.Bacc(None, target_bir_lowering=False)

    xs = nc.dram_tensor("xs", [BPC, CIN, H, W], f32, kind="ExternalInput").ap()
    w1s = nc.dram_tensor("w1s", [128, 2, MID], sdt, kind="ExternalInput").ap()
    w2p = nc.dram_tensor("w2p", [128, BPC, 3, MID], sdt, kind="ExternalInput").ap()
    w2s = nc.dram_tensor("w2s", [MID, BPC, 3, MID], sdt, kind="ExternalInput").ap()
    w3s = nc.dram_tensor("w3s", [MID, 2, 128], sdt, kind="ExternalInput").ap()
    iden = nc.dram_tensor("iden", [128, 128], sdt, kind="ExternalInput").ap()
    tvec = nc.dram_tensor("tvec", [128, 4], f32, kind="ExternalInput").ap()
    out = nc.dram_tensor("out", [BPC, CIN, H, W], f32, kind="ExternalOutput").ap()

    with tile.TileContext(nc) as tc:
        with (
            tc.tile_pool(name="consts", bufs=1) as consts,
            tc.tile_pool(name="xslab", bufs=2) as xpool,
            tc.tile_pool(name="o1p", bufs=1) as o1pool,
            tc.tile_pool(name="out2", bufs=2) as o2pool,
            tc.tile_pool(name="oslab", bufs=2) as opool,
            tc.tile_pool(name="psA", bufs=2, space="PSUM") as psA,
            tc.tile_pool(name="psB", bufs=2, space="PSUM") as psB,
            tc.tile_pool(name="psC", bufs=2, space="PSUM") as psC,
        ):
            w1t = consts.tile([128, 2, MID], sdt)
            w2pt = consts.tile([128, BPC, 3, MID], sdt)
            w2st = consts.tile([MID, BPC, 3, MID], sdt)
            w3t = consts.tile([MID, 2, 128], sdt)
            idt = consts.tile([128, 128], sdt)
            tvt = consts.tile([128, 4], f32)
            nc.sync.dma_start(out=w1t[:], in_=w1s[:])
            nc.sync.dma_start(out=w2pt[:], in_=w2p[:])
            nc.sync.dma_start(out=w2st[:], in_=w2s[:])
            nc.sync.dma_start(out=w3t[:], in_=w3s[:])
            nc.sync.dma_start(out=idt[:], in_=iden[:])
            nc.sync.dma_start(out=tvt[:], in_=tvec[:])
            t1v = tvt[0:MID, 0:1]
            t2v = tvt[0:MID, 1:2]

            # two persistent padded-out1 buffers (manual double buffering so the
            # zeroed border columns survive across slabs)
            o1tiles = [
                o1pool.tile([128, PROWS, PW], sdt, tag=f"o1p{i}", name=f"o1p{i}")
                for i in range(2)
            ]
            for t in o1tiles:
                nc.vector.memset(t[:], 0.0)

            slab_idx = 0
            for _rep in range(repeat):
                for b in range(BPC):
                    for s in range(NSLABS):
                        r0 = s * SLAB
                        o1t = o1tiles[slab_idx % 2]
                        slab_idx += 1

                        # ---- load x slab (serves conv1 and the residual) ----
                        xlo = max(r0 - 1, 0)
                        xhi = min(r0 + SLAB + 1, H)
                        nx = xhi - xlo
                        xt = xpool.tile([128, 2, PROWS * W], sdt, tag="xt", name="xt")
                        src = xs[b, :, xlo:xhi, :].rearrange(
                            "(k p) h w -> p k (h w)", p=128
                        )
                        if cast_load:
                            nc.gpsimd.dma_start(out=xt[:, :, : nx * W], in_=src)
                        else:
                            nc.sync.dma_start(out=xt[:, :, : nx * W], in_=src)

                        # ---- stage A: conv1 (1x1, 256->64) + bn1 + relu ----
                        if s == 0:
                            nc.vector.memset(o1t[0:MID, 0, :], 0.0)
                        if s == NSLABS - 1:
                            nc.vector.memset(o1t[0:MID, PROWS - 1, :], 0.0)
                        for h0, nr in _chunks(xlo, xhi, 4):
                            n = nr * W
                            psa = psA.tile([MID, 512], f32, tag="psA", name="psa")
                            for k in range(2):
                                nc.tensor.matmul(
                                    psa[:, :n],
                                    lhsT=mm_ap(w1t[:, k, :]),
                                    rhs=mm_ap(
                                        xt[:, k, (h0 - xlo) * W : (h0 - xlo) * W + n]
                                    ),
                                    start=(k == 0),
                                    stop=(k == 1),
                                )
                            lr = h0 + 1 - r0
                            nc.scalar.activation(
                                o1t[0:MID, lr : lr + nr, 1 : 1 + W],
                                psa[:, :n],
                                Act.Relu,
                                bias=t1v,
                            )
                            glo = max(h0, r0)
                            ghi = min(h0 + nr, r0 + SLAB)
                            if ghi > glo:
                                g0 = glo - r0
                                gn = ghi - glo
                                nc.vector.tensor_scalar(
                                    o1t[64 : 64 + MID, g0 : g0 + gn, 1 : 1 + W],
                                    psa[:, (glo - h0) * W : (ghi - h0) * W],
                                    t1v,
                                    0.0,
                                    Alu.add,
                                    Alu.max,
                                )

                        # ---- stage B: conv3x3 (64->64) * gate + bn2 + relu ----
                        o2t = o2pool.tile([MID, SLAB * W], sdt, tag="out2", name="o2t")
                        for j in range(SLAB // 4):
                            h0 = r0 + 4 * j
                            lr = h0 - r0
                            n = 4 * W
                            psb = psB.tile([MID, 512], f32, tag="psB", name="psb")
                            for kw in range(3):
                                nc.tensor.matmul(
                                    psb[:, :n],
                                    lhsT=mm_ap(w2pt[:, b, kw, :]),
                                    rhs=mm_ap(o1t[:, lr : lr + 4, kw : kw + W]),
                                    start=(kw == 0),
                                    stop=False,
                                )
                            for kw in range(3):
                                nc.tensor.matmul(
                                    psb[:, :n],
                                    lhsT=mm_ap(w2st[:, b, kw, :]),
                                    rhs=mm_ap(
                                        o1t[0:MID, lr + 2 : lr + 6, kw : kw + W]
                                    ),
                                    start=False,
                                    stop=(kw == 2),
                                )
                            nc.scalar.activation(
                                o2t[:, j * n : (j + 1) * n],
                                psb[:, :n],
                                Act.Relu,
                                bias=t2v,
                            )

                        # ---- stage C: conv1x1 (64->256) + bn3 + x + relu ----
                        # psc pairs two adjacent chunks (2 PSUM banks); the
                        # residual x rides the PE as an identity matmul.
                        ot = opool.tile([128, 2, SLAB * W], f32, tag="ot", name="ot")
                        evac_flip = 0
                        for m in range(2):
                            for j0 in range(0, SLAB // 4, 2):
                                npair = min(2, SLAB // 4 - j0)
                                psc = psC.tile(
                                    [128, 2, 512], f32, tag="psC", name="psc"
                                )
                                for jj in range(npair):
                                    j = j0 + jj
                                    off = j * 4 * W
                                    n = 4 * W
                                    xoff = (r0 + 4 * j - xlo) * W
                                    nc.tensor.matmul(
                                        psc[:, jj, :n],
                                        lhsT=mm_ap(w3t[:, m, :]),
                                        rhs=mm_ap(o2t[:, off : off + n]),
                                        start=True,
                                        stop=False,
                                    )
                                    nc.tensor.matmul(
                                        psc[:, jj, :n],
                                        lhsT=mm_ap(idt[:]),
                                        rhs=mm_ap(xt[:, m, xoff : xoff + n]),
                                        start=False,
                                        stop=True,
                                    )
                                off = j0 * 4 * W
                                n = npair * 4 * W
                                t3v = tvt[:, 2 + m : 3 + m]
                                dst = ot[:, m, off : off + n]
                                src_ = psc[:, :npair, : 4 * W]
                                if evac_flip % 2 == 0:
                                    nc.scalar.activation(dst, src_, Act.Relu, bias=t3v)
                                else:
                                    nc.vector.tensor_scalar(
                                        dst, src_, t3v, 0.0, Alu.add, Alu.max
                                    )
                                evac_flip += 1
                        for m in range(2):
                            dstd = out[b, m * 128 : (m + 1) * 128, r0 : r0 + SLAB, :]
                            nc.sync.dma_start(
                                out=dstd.rearrange("c h w -> c (h w)"),
                                in_=ot[:, m, :],
                            )

    nc.compile()
    return nc


def _get_nc(repeat=1, mm_dtype=None):
    key = ("nc", repeat, mm_dtype or MM_DTYPE)
    if key not in _cache:
        _cache[key] = build_nc(repeat, mm_dtype)
    return _cache[key]


def _np_dtype(mm_dtype):
    if mm_dtype in ("float32", "float32r"):
        return np.float32
    if mm_dtype == "float16":
        return np.float16
    import ml_dtypes

    return ml_dtypes.bfloat16


def _prep_inputs(x, embeddings, w1, bn1_g, bn1_b, bn1_m, bn1_v,
                 w2, bn2_g, bn2_b, bn2_m, bn2_v,
                 w3, bn3_g, bn3_b, bn3_m, bn3_v, gate_w, gate_b,
                 mm_dtype=None):
    """Host-side folding of BN / gate into conv weights. Returns
    (per-core input maps, gate)."""
    mm_dtype = mm_dtype or MM_DTYPE
    wdt = _np_dtype(mm_dtype)
    f = np.float32
    x = np.asarray(x, f)
    s1 = (bn1_g / np.sqrt(bn1_v + EPS)).astype(np.float64)
    t1 = (bn1_b - bn1_m * s1).astype(f)
    s2 = (bn2_g / np.sqrt(bn2_v + EPS)).astype(np.float64)
    t2 = (bn2_b - bn2_m * s2).astype(f)
    s3 = (bn3_g / np.sqrt(bn3_v + EPS)).astype(np.float64)
    t3 = (bn3_b - bn3_m * s3).astype(f)

    gate = np.maximum(
        embeddings.astype(np.float64) @ gate_w.astype(np.float64).T
        + gate_b.astype(np.float64),
        0.0,
    )  # [B, MID]

    # conv1 lhsT with bn1 scale folded: w1s[ci, k, co] = w1[co, k*128+ci] * s1[co]
    w1f = w1[:, :, 0, 0].astype(np.float64) * s1[:, None]          # [64, 256]
    w1s = np.ascontiguousarray(
        w1f.T.reshape(2, 128, MID).transpose(1, 0, 2)
    ).astype(wdt)                                                   # [128, 2, 64]

    # conv3 lhsT with bn3 scale folded: w3s[ci, m, co] = w3[m*128+co, ci] * s3[...]
    w3f = w3[:, :, 0, 0].astype(np.float64) * s3[:, None]          # [256, 64]
    w3s = np.ascontiguousarray(
        w3f.reshape(2, 128, MID).transpose(2, 0, 1)
    ).astype(wdt)                                                   # [64, 2, 128]

    iden = np.eye(128, dtype=wdt)

    tvec = np.zeros((128, 4), f)
    tvec[:MID, 0] = t1
    tvec[:MID, 1] = t2
    tvec[:, 2] = t3[:128]
    tvec[:, 3] = t3[128:]

    # per-image folded conv2 weights: w2[co, ci, kh, kw] * gate[b, co] * s2[co]
    w2_64 = w2.astype(np.float64)                                   # [64,64,3,3]
    in_maps = []
    for c in range(N_CORES):
        bsl = slice(c * BPC, (c + 1) * BPC)
        gs2 = gate[bsl] * s2[None, :]                               # [BPC, 64]
        wk = w2_64[None, :, :, :, :] * gs2[:, :, None, None, None]  # [b,co,ci,kh,kw]
        w2p = np.empty((128, BPC, 3, MID), wdt)
        w2p[:MID] = wk[:, :, :, 0, :].transpose(2, 0, 3, 1)         # [ci,b,kw,co]
        w2p[MID:] = wk[:, :, :, 1, :].transpose(2, 0, 3, 1)
        w2s_ = np.ascontiguousarray(
            wk[:, :, :, 2, :].transpose(2, 0, 3, 1)
        ).astype(wdt)                                               # [64,b,kw,co]
        in_maps.append(
            {
                "xs": np.ascontiguousarray(x[bsl]),
                "w1s": w1s,
                "w2p": np.ascontiguousarray(w2p),
                "w2s": w2s_,
                "w3s": w3s,
                "iden": iden,
                "tvec": tvec,
            }
        )
    return in_maps, gate.astype(f)


def kernel(**inputs):
    from concourse.bass_utils import run_bass_kernel_spmd

    in_maps, gate = _prep_inputs(**inputs)
    nc = _get_nc()
    res = run_bass_kernel_spmd(nc, in_maps, core_ids=list(range(N_CORES)))
    out = np.concatenate([res.results[c]["out"] for c in range(N_CORES)], axis=0)
    return (out, gate)
